# revision 12
# baseline (speedup 1.0000x reference)
"""GraphTransformer (2x PyG TransformerConv + linear) on 8 trn2 NeuronCores.

v2: algebraic restructure + bf16 + dma_gather bulk gathers.

Key ideas vs v1:
  -  alpha_e,h = q_h[dst].k_h[src] = u_h[dst] . x[src] with
     u = x @ (Wq_h Wk_h^T): only the 64-wide x row is gathered per edge
     (bf16, padded to 128 cols = 256B rows for dma_gather).
  -  sum_e a_e v_h[src_e] = (sum_e a_e x[src_e]) @ Wv_h: Wv applied once
     per node in the dense pass, never per edge.
  -  dma_gather (one Q7 instruction per ~1-2K rows) replaces per-128-row
     indirect DMAs.  int16 index limit handled by a low/high src split:
     each block keeps edges with src < 32768 in subtiles 0..4 and the
     rest in subtiles 5..7; pass A gathers from tab, pass B from
     tab[32768:].
  -  all PE matmuls in bf16; M and M^T one-hot segment matrices are
     host-built bf16 and DMAd (no on-device transposes).
  -  "holey" slot-major layout (node -> fixed block slot) so the
     projection, stage and output tensors are all read/written with
     direct DMAs; the host compacts/expands between launches.
Two launches: L1 (proj + conv1 + layer-2 projection), L2 (conv2 + final
linear).  Host does index prep, bf16 casts and table redistribution.
"""
import sys

sys.path.insert(0, "/opt/trn_rl_repo")
import numpy as np
import ml_dtypes
import concourse.bass as bass
import concourse.bacc as bacc
import concourse.tile as tile
from concourse import mybir
from concourse import library_config
from concourse.bass_utils import run_bass_kernel_spmd

F32 = mybir.dt.float32
BF16 = mybir.dt.bfloat16
I16 = mybir.dt.int16
BF = ml_dtypes.bfloat16

NCORES = 8
NNODE = 50000
SPLIT = 32768
DPB = 64                 # dst slots per block
LO_T, HI_T = 5, 3        # low/high-src subtiles per block
TS = LO_T + HI_T         # 8 subtiles per block, 16 per pair-iteration
LOE, HIE = LO_T * 128, HI_T * 128

_built = {}


def _bc(ap, p):
    return bass.AP(tensor=ap.tensor, offset=ap.offset, ap=[[0, p]] + list(ap.ap[1:]))


def _build_L1(B2):
    """proj (u|sk) + conv1 block loop + dense pass emitting kvqs2."""
    NS = B2 * 128
    nc = bacc.Bacc(num_swdge_queues=4)
    xT = nc.declare_dram_parameter("xT", [64, NS], BF16, isOutput=False)
    W1 = nc.declare_dram_parameter("W1", [64, 512], BF16, isOutput=False)
    b1 = nc.declare_dram_parameter("b1", [1, 512], F32, isOutput=False)
    xgA = nc.declare_dram_parameter("xgA", [NNODE, 128], BF16, isOutput=False)
    xgB = nc.declare_dram_parameter("xgB", [NNODE - SPLIT, 128], BF16, isOutput=False)
    idxA = nc.declare_dram_parameter("idxA", [B2, 128, 2 * LOE // 16], I16, isOutput=False)
    idxB = nc.declare_dram_parameter("idxB", [B2, 128, 2 * HIE // 16], I16, isOutput=False)
    Mb = nc.declare_dram_parameter("Mb", [B2, 128, 16 * DPB], BF16, isOutput=False)
    MTb = nc.declare_dram_parameter("MTb", [B2, 64, 16 * 128], BF16, isOutput=False)
    WvBD = nc.declare_dram_parameter("WvBD", [256, 256], BF16, isOutput=False)
    W2 = nc.declare_dram_parameter("W2", [256, 256], BF16, isOutput=False)
    b2 = nc.declare_dram_parameter("b2", [1, 256], F32, isOutput=False)
    idn = nc.declare_dram_parameter("idn", [128, 128], BF16, isOutput=False)
    outt = nc.declare_dram_parameter("outt", [NS, 256], BF16, isOutput=True)
    proj = nc.dram_tensor("proj", [NS, 512], BF16)
    stage = nc.dram_tensor("stage", [NS, 260], BF16)

    # subtile t -> (block parity, is_low);  layout: 0-4 loB, 5-9 loB1,
    # 10-12 hiB, 13-15 hiB1
    blk_of = [0] * LO_T + [1] * LO_T + [0] * HI_T + [1] * HI_T

    with tile.TileContext(nc) as tc:
        nc.gpsimd.load_library(library_config.mlp)
        with tc.tile_pool(name="one", bufs=1) as one:
            W1t = one.tile([64, 512], BF16)
            nc.sync.dma_start(out=W1t[:], in_=W1[:])
            b1t = one.tile([128, 512], F32)
            nc.sync.dma_start(out=b1t[:], in_=_bc(b1[:], 128))
            Wvt = one.tile([128, 2, 256], BF16, name="wvt")
            W2t = one.tile([128, 2, 256], BF16, name="w2t")
            for k in range(2):
                nc.sync.dma_start(out=Wvt[:, k, :], in_=WvBD[k * 128:(k + 1) * 128, :])
                nc.sync.dma_start(out=W2t[:, k, :], in_=W2[k * 128:(k + 1) * 128, :])
            b2t = one.tile([128, 256], F32)
            nc.sync.dma_start(out=b2t[:], in_=_bc(b2[:], 128))
            ident = one.tile([128, 128], BF16)
            nc.sync.dma_start(out=ident[:], in_=idn[:])

            # ---- phase P: proj = x @ [A|Ws] + [bu|bs'] (slot-major) ----
            with tc.tile_pool(name="psb", bufs=3) as sb, \
                 tc.tile_pool(name="pps", bufs=2, space="PSUM") as ps:
                for j in range(B2):
                    xt = sb.tile([64, 128], BF16, tag="xt")
                    nc.sync.dma_start(out=xt[:], in_=xT[:, j * 128:(j + 1) * 128])
                    pp = ps.tile([128, 512], F32, tag="pp")
                    nc.tensor.matmul(out=pp[:], lhsT=xt[:], rhs=W1t[:],
                                     start=True, stop=True)
                    pt = sb.tile([128, 512], BF16, tag="pt")
                    nc.vector.tensor_add(pt[:], pp[:], b1t[:])
                    nc.sync.dma_start(out=proj[j * 128:(j + 1) * 128, :], in_=pt[:])

            # ---- block-pair loop ----
            with tc.tile_pool(name="sb", bufs=2) as sb, \
                 tc.tile_pool(name="qps", bufs=2, space="PSUM") as qps, \
                 tc.tile_pool(name="aps", bufs=2, space="PSUM") as aps:
                for i in range(B2):
                    iA = sb.tile([128, 2 * LOE // 16], I16, tag="iA")
                    nc.sync.dma_start(out=iA[:], in_=idxA[i])
                    iB = sb.tile([128, 2 * HIE // 16], I16, tag="iB")
                    nc.sync.dma_start(out=iB[:], in_=idxB[i])
                    xs = sb.tile([128, 16, 128], BF16, tag="xs")
                    nc.gpsimd.dma_gather(
                        xs[:, 0:2 * LO_T, :], xgA[:], iA[:], 2 * LOE, 2 * LOE,
                        128, single_packet=False, queue_num=(2 * i) % 4)
                    nc.gpsimd.dma_gather(
                        xs[:, 2 * LO_T:16, :], xgB[:], iB[:], 2 * HIE, 2 * HIE,
                        128, single_packet=False, queue_num=(2 * i + 1) % 4)
                    Mt = sb.tile([128, 16, DPB], BF16, tag="Mt")
                    nc.sync.dma_start(out=Mt[:], in_=Mb[i])
                    MTt = sb.tile([64, 16, 128], BF16, tag="MTt")
                    nc.sync.dma_start(out=MTt[:], in_=MTb[i])
                    qr = [sb.tile([64, 256], BF16, tag=f"qr{h}", name=f"qr{h}")
                          for h in range(2)]
                    for h in range(2):
                        nc.sync.dma_start(
                            out=qr[h][:],
                            in_=proj[i * 128 + h * 64:i * 128 + (h + 1) * 64, 0:256])
                    vwe = sb.tile([128, 16, 260], BF16, tag="vwe")
                    prod = sb.tile([128, 16, 4, 64], BF16, tag="prod")
                    qepb = sb.tile([128, 16, 256], BF16, tag="qepb")
                    alph = sb.tile([128, 16, 4], BF16, tag="alph")
                    E4 = sb.tile([128, 16, 4, 64], BF16, tag="E4")
                    for g in range(4):
                        qep = qps.tile([128, 4, 256], F32, tag="qep")
                        for k in range(4):
                            t = g * 4 + k
                            nc.tensor.matmul(out=qep[:, k, :],
                                             lhsT=MTt[:, t, :], rhs=qr[blk_of[t]][:],
                                             start=True, stop=True)
                        sl = slice(g * 4, (g + 1) * 4)
                        nc.scalar.activation(qepb[:, sl], qep[:],
                                             mybir.ActivationFunctionType.Copy)
                        nc.vector.tensor_mul(
                            prod[:, sl],
                            xs[:, sl, 0:64].unsqueeze(2).to_broadcast([128, 4, 4, 64]),
                            qepb[:, sl].rearrange("p k (h d) -> p k h d", h=4))
                        with nc.allow_low_precision(reason="O(1) attn scores"):
                            nc.vector.reduce_sum(
                                out=alph[:, sl], in_=prod[:, sl],
                                axis=mybir.AxisListType.X)
                    nc.scalar.activation(
                        E4[:], alph[:].unsqueeze(3).to_broadcast([128, 16, 4, 64]),
                        mybir.ActivationFunctionType.Exp, scale=0.125)
                    nc.scalar.activation(vwe[:, :, 256:260], alph[:],
                                         mybir.ActivationFunctionType.Exp,
                                         scale=0.125)
                    nc.vector.tensor_mul(
                        vwe[:, :, 0:256].rearrange("p t (h d) -> p t h d", h=4),
                        xs[:, :, 0:64].unsqueeze(2).to_broadcast([128, 16, 4, 64]),
                        E4[:])
                    agg = aps.tile([128, 260], F32, tag="agg")
                    last_t = {0: LO_T * 2 + HI_T - 1, 1: 15}
                    first_t = {0: 0, 1: LO_T}
                    for t in range(16):
                        b = blk_of[t]
                        nc.tensor.matmul(out=agg[b * 64:(b + 1) * 64, :],
                                         lhsT=Mt[:, t, :], rhs=vwe[:, t, :],
                                         start=(t == first_t[b]),
                                         stop=(t == last_t[b]))
                    st = sb.tile([128, 260], BF16, tag="st")
                    nc.vector.tensor_copy(st[:], agg[:])
                    nc.sync.dma_start(out=stage[i * 128:(i + 1) * 128, :], in_=st[:])

            # ---- dense pass: normalize, Wv, skip, relu, W2 ----
            with tc.tile_pool(name="db", bufs=3) as sb, \
                 tc.tile_pool(name="dps", bufs=2, space="PSUM") as ps:
                for i in range(B2):
                    pre = sb.tile([128, 260], BF16, tag="pre")
                    nc.sync.dma_start(out=pre[:], in_=stage[i * 128:(i + 1) * 128, :])
                    sc = sb.tile([128, 4], F32, tag="sc")
                    nc.vector.tensor_scalar_max(sc[:], pre[:, 256:260], 1e-30)
                    rs = sb.tile([128, 4], F32, tag="rs")
                    nc.vector.reciprocal(rs[:], sc[:])
                    zn = sb.tile([128, 256], BF16, tag="zn")
                    nc.vector.tensor_mul(
                        zn[:].rearrange("p (h d) -> p h d", h=4),
                        pre[:, 0:256].rearrange("p (h d) -> p h d", h=4),
                        rs[:].unsqueeze(2).to_broadcast([128, 4, 64]))
                    znTp = ps.tile([128, 256], BF16, tag="znTp")
                    for k in range(2):
                        nc.tensor.transpose(out=znTp[:, k * 128:(k + 1) * 128],
                                            in_=zn[:, k * 128:(k + 1) * 128],
                                            identity=ident[:])
                    znT = sb.tile([128, 256], BF16, tag="znT")
                    nc.vector.tensor_copy(znT[:], znTp[:])
                    h1p = ps.tile([128, 256], F32, tag="h1p")
                    for k in range(2):
                        nc.tensor.matmul(out=h1p[:], lhsT=znT[:, k * 128:(k + 1) * 128],
                                         rhs=Wvt[:, k, :],
                                         start=(k == 0), stop=(k == 1))
                    skt = sb.tile([128, 256], BF16, tag="skt")
                    nc.sync.dma_start(out=skt[:],
                                      in_=proj[i * 128:(i + 1) * 128, 256:512])
                    hb = sb.tile([128, 256], BF16, tag="hb")
                    nc.vector.tensor_add(hb[:], h1p[:], skt[:])
                    nc.scalar.activation(hb[:], hb[:],
                                         mybir.ActivationFunctionType.Relu)
                    hTp = ps.tile([128, 256], BF16, tag="hTp")
                    for k in range(2):
                        nc.tensor.transpose(out=hTp[:, k * 128:(k + 1) * 128],
                                            in_=hb[:, k * 128:(k + 1) * 128],
                                            identity=ident[:])
                    hT = sb.tile([128, 256], BF16, tag="hT")
                    nc.vector.tensor_copy(hT[:], hTp[:])
                    o2p = ps.tile([128, 256], F32, tag="o2p")
                    for k in range(2):
                        nc.tensor.matmul(out=o2p[:], lhsT=hT[:, k * 128:(k + 1) * 128],
                                         rhs=W2t[:, k, :],
                                         start=(k == 0), stop=(k == 1))
                    ot = sb.tile([128, 256], BF16, tag="ot")
                    nc.vector.tensor_add(ot[:], o2p[:], b2t[:])
                    nc.sync.dma_start(out=outt[i * 128:(i + 1) * 128, :], in_=ot[:])
    nc.finalize()
    return nc


def _build_L2(B2):
    """conv2 (1 head) + final linear."""
    NS = B2 * 128
    nc = bacc.Bacc(num_swdge_queues=4)
    p2 = nc.declare_dram_parameter("p2", [NS, 128], BF16, isOutput=False)   # q2|sk2
    kvA = nc.declare_dram_parameter("kvA", [NNODE, 128], BF16, isOutput=False)
    kvB = nc.declare_dram_parameter("kvB", [NNODE - SPLIT, 128], BF16, isOutput=False)
    idxA = nc.declare_dram_parameter("idxA", [B2, 128, 2 * LOE // 16], I16, isOutput=False)
    idxB = nc.declare_dram_parameter("idxB", [B2, 128, 2 * HIE // 16], I16, isOutput=False)
    Mb = nc.declare_dram_parameter("Mb", [B2, 128, 16 * DPB], BF16, isOutput=False)
    MTb = nc.declare_dram_parameter("MTb", [B2, 64, 16 * 128], BF16, isOutput=False)
    Wl = nc.declare_dram_parameter("Wl", [64, 2], BF16, isOutput=False)
    bl = nc.declare_dram_parameter("bl", [1, 2], F32, isOutput=False)
    idn = nc.declare_dram_parameter("idn", [128, 128], BF16, isOutput=False)
    outf = nc.declare_dram_parameter("outf", [NS, 2], F32, isOutput=True)
    stage = nc.dram_tensor("stage2", [NS, 65], BF16)

    blk_of = [0] * LO_T + [1] * LO_T + [0] * HI_T + [1] * HI_T

    with tile.TileContext(nc) as tc:
        nc.gpsimd.load_library(library_config.mlp)
        with tc.tile_pool(name="one", bufs=1) as one:
            Wlt = one.tile([64, 2], BF16)
            nc.sync.dma_start(out=Wlt[:], in_=Wl[:])
            blt = one.tile([128, 2], F32)
            nc.sync.dma_start(out=blt[:], in_=_bc(bl[:], 128))
            ident = one.tile([128, 128], BF16)
            nc.sync.dma_start(out=ident[:], in_=idn[:])

            with tc.tile_pool(name="sb", bufs=2) as sb, \
                 tc.tile_pool(name="qps", bufs=2, space="PSUM") as qps, \
                 tc.tile_pool(name="aps", bufs=2, space="PSUM") as aps:
                for i in range(B2):
                    iA = sb.tile([128, 2 * LOE // 16], I16, tag="iA")
                    nc.sync.dma_start(out=iA[:], in_=idxA[i])
                    iB = sb.tile([128, 2 * HIE // 16], I16, tag="iB")
                    nc.sync.dma_start(out=iB[:], in_=idxB[i])
                    kvt = sb.tile([128, 16, 128], BF16, tag="kvt")
                    nc.gpsimd.dma_gather(
                        kvt[:, 0:2 * LO_T, :], kvA[:], iA[:], 2 * LOE, 2 * LOE,
                        128, single_packet=False, queue_num=(2 * i) % 4)
                    nc.gpsimd.dma_gather(
                        kvt[:, 2 * LO_T:16, :], kvB[:], iB[:], 2 * HIE, 2 * HIE,
                        128, single_packet=False, queue_num=(2 * i + 1) % 4)
                    Mt = sb.tile([128, 16, DPB], BF16, tag="Mt")
                    nc.sync.dma_start(out=Mt[:], in_=Mb[i])
                    MTt = sb.tile([64, 16, 128], BF16, tag="MTt")
                    nc.sync.dma_start(out=MTt[:], in_=MTb[i])
                    qr = [sb.tile([64, 128], BF16, tag=f"qr{h}", name=f"qr{h}")
                          for h in range(2)]
                    for h in range(2):
                        nc.sync.dma_start(
                            out=qr[h][:],
                            in_=p2[i * 128 + h * 64:i * 128 + (h + 1) * 64, :])
                    vwe = sb.tile([128, 16, 65], BF16, tag="vwe")
                    prod = sb.tile([128, 16, 64], BF16, tag="prod")
                    qepb = sb.tile([128, 16, 64], BF16, tag="qepb")
                    alph = sb.tile([128, 16, 1], BF16, tag="alph")
                    E4 = sb.tile([128, 16, 64], BF16, tag="E4")
                    qep = qps.tile([128, 16, 64], F32, tag="qep")
                    for t in range(16):
                        nc.tensor.matmul(out=qep[:, t, :], lhsT=MTt[:, t, :],
                                         rhs=qr[blk_of[t]][:, 0:64],
                                         start=True, stop=True)
                    nc.scalar.activation(qepb[:], qep[:],
                                         mybir.ActivationFunctionType.Copy)
                    nc.vector.tensor_mul(prod[:], kvt[:, :, 0:64], qepb[:])
                    with nc.allow_low_precision(reason="O(1) attn scores"):
                        nc.vector.reduce_sum(out=alph[:], in_=prod[:],
                                             axis=mybir.AxisListType.X)
                    nc.scalar.activation(
                        E4[:], alph[:].to_broadcast([128, 16, 64]),
                        mybir.ActivationFunctionType.Exp, scale=0.125)
                    nc.scalar.activation(vwe[:, :, 64:65], alph[:],
                                         mybir.ActivationFunctionType.Exp,
                                         scale=0.125)
                    nc.vector.tensor_mul(
                        vwe[:, :, 0:64], kvt[:, :, 64:128], E4[:])
                    agg = aps.tile([128, 65], F32, tag="agg")
                    last_t = {0: LO_T * 2 + HI_T - 1, 1: 15}
                    first_t = {0: 0, 1: LO_T}
                    for t in range(16):
                        b = blk_of[t]
                        nc.tensor.matmul(out=agg[b * 64:(b + 1) * 64, :],
                                         lhsT=Mt[:, t, :], rhs=vwe[:, t, :],
                                         start=(t == first_t[b]),
                                         stop=(t == last_t[b]))
                    st = sb.tile([128, 65], BF16, tag="st")
                    nc.vector.tensor_copy(st[:], agg[:])
                    nc.sync.dma_start(out=stage[i * 128:(i + 1) * 128, :], in_=st[:])

            with tc.tile_pool(name="db", bufs=3) as sb, \
                 tc.tile_pool(name="dps", bufs=2, space="PSUM") as ps:
                for i in range(B2):
                    pre = sb.tile([128, 65], BF16, tag="pre")
                    nc.sync.dma_start(out=pre[:], in_=stage[i * 128:(i + 1) * 128, :])
                    sc = sb.tile([128, 1], F32, tag="sc")
                    nc.vector.tensor_scalar_max(sc[:], pre[:, 64:65], 1e-30)
                    rs = sb.tile([128, 1], F32, tag="rs")
                    nc.vector.reciprocal(rs[:], sc[:])
                    sk2 = sb.tile([128, 64], BF16, tag="sk2")
                    nc.sync.dma_start(out=sk2[:],
                                      in_=p2[i * 128:(i + 1) * 128, 64:128])
                    h2 = sb.tile([128, 64], BF16, tag="h2")
                    nc.vector.tensor_mul(h2[:], pre[:, 0:64],
                                         rs[:].to_broadcast([128, 64]))
                    nc.vector.tensor_add(h2[:], h2[:], sk2[:])
                    nc.scalar.activation(h2[:], h2[:],
                                         mybir.ActivationFunctionType.Relu)
                    hTp = ps.tile([64, 128], BF16, tag="hTp")
                    nc.tensor.transpose(out=hTp[:], in_=h2[:], identity=ident[:])
                    hT = sb.tile([64, 128], BF16, tag="hT")
                    nc.vector.tensor_copy(hT[:], hTp[:])
                    op = ps.tile([128, 2], F32, tag="op")
                    nc.tensor.matmul(out=op[:], lhsT=hT[:], rhs=Wlt[:],
                                     start=True, stop=True)
                    oo = sb.tile([128, 2], F32, tag="oo")
                    nc.vector.tensor_add(oo[:], op[:], blt[:])
                    nc.sync.dma_start(out=outf[i * 128:(i + 1) * 128, :], in_=oo[:])
    nc.finalize()
    return nc


def _wrap_idx(v, n):
    """[n] int16 -> [128, n//16] dma_gather layout (16-wrap, 8x replicated)."""
    assert v.shape[0] == n and n % 16 == 0
    m = v.reshape(n // 16, 16).T                    # [16, n//16]
    return np.tile(m, (8, 1)).astype(np.int16)      # [128, n//16]


def _prep(edge_index):
    """Sort edges by dst, split low/high src, pack into block pairs."""
    src = np.ascontiguousarray(edge_index[0]).astype(np.int64)
    dst = np.ascontiguousarray(edge_index[1]).astype(np.int64)
    E = src.shape[0]
    # sort by dst; within each dst, low srcs (< SPLIT) first
    comb = np.lexsort((src >= SPLIT, dst))
    s_sorted = src[comb]
    d_sorted = dst[comb]
    deg = np.bincount(d_sorted, minlength=NNODE)
    nlo = np.bincount(d_sorted[s_sorted < SPLIT], minlength=NNODE)
    nhi = deg - nlo
    cume = np.concatenate([[0], np.cumsum(deg)])
    targets = [round(E * c / NCORES) for c in range(1, NCORES)]
    nb = [0] + [int(np.searchsorted(cume, t)) for t in targets] + [NNODE]

    cores = []
    for c in range(NCORES):
        n0, n1 = nb[c], nb[c + 1]
        blocks, cur, lo, hi = [], [], 0, 0
        for n in range(n0, n1):
            gl, gh = int(nlo[n]), int(nhi[n])
            assert gl <= LOE and gh <= HIE
            if len(cur) >= DPB or lo + gl > LOE or hi + gh > HIE:
                blocks.append(cur)
                cur, lo, hi = [], 0, 0
            cur.append(n)
            lo += gl
            hi += gh
        if cur:
            blocks.append(cur)
        if len(blocks) % 2:
            blocks.append([])
        cores.append((n0, n1, blocks))
    B2 = max(len(cb) // 2 for _, _, cb in cores)

    per_core = []
    for c in range(NCORES):
        n0, n1, blocks = cores[c]
        idxA = np.zeros((B2, 2 * LOE), np.int64)
        idxB = np.full((B2, 2 * HIE), SPLIT, np.int64)
        Mb = np.zeros((B2, 128, 16 * DPB), BF)
        MTb = np.zeros((B2, 64, 16 * 128), BF)
        slot_node = np.full((B2 * 128,), -1, np.int64)   # slot -> node
        for bi, nodes in enumerate(blocks):
            i, half = bi // 2, bi % 2
            lo_base = half * LOE                 # offset into idxA[i]
            hi_base = half * HIE
            elo, ehi = 0, 0
            for sl, n in enumerate(nodes):
                slot_node[i * 128 + half * 64 + sl] = n
                lo0, hi0 = cume[n], cume[n] + nlo[n]
                for k in range(int(nlo[n])):
                    idxA[i, lo_base + elo + k] = s_sorted[lo0 + k]
                for k in range(int(nhi[n])):
                    idxB[i, hi_base + ehi + k] = s_sorted[hi0 + k]
                elo += int(nlo[n])
                ehi += int(nhi[n])
            # build M/MT for this block from the slot runs
            elo, ehi = 0, 0
            for sl, n in enumerate(nodes):
                for k in range(int(nlo[n])):
                    e = elo + k                   # 0..LOE-1 within block
                    t = half * LO_T + e // 128    # subtile index (0..9)
                    p = e % 128
                    Mb[i, p, t * DPB + sl] = 1.0
                    MTb[i, sl, t * 128 + p] = 1.0
                for k in range(int(nhi[n])):
                    e = ehi + k
                    t = 2 * LO_T + half * HI_T + e // 128   # 10..15
                    p = e % 128
                    Mb[i, p, t * DPB + sl] = 1.0
                    MTb[i, sl, t * 128 + p] = 1.0
                elo += int(nlo[n])
                ehi += int(nhi[n])
        idxBw = np.zeros((B2, 128, 2 * HIE // 16), np.int16)
        idxAw = np.zeros((B2, 128, 2 * LOE // 16), np.int16)
        for i in range(B2):
            idxAw[i] = _wrap_idx(idxA[i].astype(np.int16), 2 * LOE)
            idxBw[i] = _wrap_idx((idxB[i] - SPLIT).astype(np.int16), 2 * HIE)
        per_core.append(dict(n0=n0, n1=n1, idxA=idxAw, idxB=idxBw, Mb=Mb,
                             MTb=MTb, slot_node=slot_node))
    return B2, per_core


def kernel(x, edge_index, Wq1, bq1, Wk1, bk1, Wv1, bv1, Ws1, bs1,
           Wq2, bq2, Wk2, bk2, Wv2, bv2, Ws2, bs2, Wl, bl):
    x = np.asarray(x, np.float32)
    B2, per_core = _prep(np.asarray(edge_index))
    NS = B2 * 128

    if ("L1", B2) not in _built:
        _built[("L1", B2)] = _build_L1(B2)
    if ("L2", B2) not in _built:
        _built[("L2", B2)] = _build_L2(B2)

    # host-folded weights
    Wq1, Wk1 = np.asarray(Wq1, np.float32), np.asarray(Wk1, np.float32)
    Wv1, Ws1 = np.asarray(Wv1, np.float32), np.asarray(Ws1, np.float32)
    bq1, bk1 = np.asarray(bq1, np.float32), np.asarray(bk1, np.float32)
    bv1, bs1 = np.asarray(bv1, np.float32), np.asarray(bs1, np.float32)
    Acat = np.zeros((64, 256), np.float32)
    bu = np.zeros((256,), np.float32)
    WvBD = np.zeros((256, 256), np.float32)
    for h in range(4):
        sl = slice(h * 64, (h + 1) * 64)
        Acat[:, sl] = Wq1[:, sl] @ Wk1[:, sl].T
        bu[sl] = bq1[sl] @ Wk1[:, sl].T
        WvBD[sl, sl] = Wv1[:, sl]
    W1cat = np.concatenate([Acat, Ws1], axis=1)          # [64, 512]
    b1cat = np.concatenate([bu, bs1 + bv1])[None, :]     # [1, 512]
    W2cat = np.concatenate([Wk2, Wv2, Wq2, Ws2], axis=1).astype(np.float32)
    b2cat = np.concatenate([bk2, bv2, bq2, bs2])[None, :].astype(np.float32)

    xg = np.zeros((NNODE, 128), BF)
    xg[:, 0:64] = x.astype(BF)
    cids = list(range(NCORES))

    in1 = []
    for c in cids:
        pc = per_core[c]
        sn = pc["slot_node"]
        xT = np.zeros((64, NS), BF)
        valid = sn >= 0
        xT[:, valid] = x[sn[valid]].T.astype(BF)
        in1.append(dict(
            xT=xT, W1=W1cat.astype(BF), b1=b1cat,
            xgA=xg, xgB=np.ascontiguousarray(xg[SPLIT:]),
            idxA=pc["idxA"], idxB=pc["idxB"], Mb=pc["Mb"], MTb=pc["MTb"],
            WvBD=WvBD.astype(BF), W2=W2cat.astype(BF), b2=b2cat,
            idn=np.eye(128, dtype=BF)))
    res1 = run_bass_kernel_spmd(_built[("L1", B2)], in1, cids)
    t1 = res1.exec_time_ns

    # compact h1-projection rows; build kv2 tables and per-core p2
    kv2 = np.zeros((NNODE, 128), BF)
    p2s = []
    for c in cids:
        pc = per_core[c]
        sn = pc["slot_node"]
        valid = sn >= 0
        o = res1.results[c]["outt"]          # [NS, 256] bf16: k2|v2|q2|sk2
        kv2[sn[valid]] = o[valid][:, 0:128]
        p2 = np.zeros((NS, 128), BF)
        p2[valid] = o[valid][:, 128:256]
        p2s.append(p2)

    in2 = []
    for c in cids:
        pc = per_core[c]
        in2.append(dict(
            p2=p2s[c], kvA=kv2, kvB=np.ascontiguousarray(kv2[SPLIT:]),
            idxA=pc["idxA"], idxB=pc["idxB"], Mb=pc["Mb"], MTb=pc["MTb"],
            Wl=np.asarray(Wl, np.float32).astype(BF),
            bl=np.asarray(bl, np.float32)[None, :],
            idn=np.eye(128, dtype=BF)))
    res2 = run_bass_kernel_spmd(_built[("L2", B2)], in2, cids)
    t2 = res2.exec_time_ns

    out = np.zeros((NNODE, 2), np.float32)
    for c in cids:
        pc = per_core[c]
        sn = pc["slot_node"]
        valid = sn >= 0
        out[sn[valid]] = res2.results[c]["outf"][valid]
    kernel.exec_times = (t1, t2)
    return out


# revision 13
# speedup vs baseline: 1.1125x; 1.1125x over previous
"""GraphTransformer (2x PyG TransformerConv + linear) on 8 trn2 NeuronCores.

v2: algebraic restructure + bf16 + dma_gather bulk gathers.

Key ideas vs v1:
  -  alpha_e,h = q_h[dst].k_h[src] = u_h[dst] . x[src] with
     u = x @ (Wq_h Wk_h^T): only the 64-wide x row is gathered per edge
     (bf16, padded to 128 cols = 256B rows for dma_gather).
  -  sum_e a_e v_h[src_e] = (sum_e a_e x[src_e]) @ Wv_h: Wv applied once
     per node in the dense pass, never per edge.
  -  dma_gather (one Q7 instruction per ~1-2K rows) replaces per-128-row
     indirect DMAs.  int16 index limit handled by a low/high src split:
     each block keeps edges with src < 32768 in subtiles 0..4 and the
     rest in subtiles 5..7; pass A gathers from tab, pass B from
     tab[32768:].
  -  all PE matmuls in bf16; M and M^T one-hot segment matrices are
     host-built bf16 and DMAd (no on-device transposes).
  -  "holey" slot-major layout (node -> fixed block slot) so the
     projection, stage and output tensors are all read/written with
     direct DMAs; the host compacts/expands between launches.
Two launches: L1 (proj + conv1 + layer-2 projection), L2 (conv2 + final
linear).  Host does index prep, bf16 casts and table redistribution.
"""
import sys

sys.path.insert(0, "/opt/trn_rl_repo")
import numpy as np
import ml_dtypes
import concourse.bass as bass
import concourse.bacc as bacc
import concourse.tile as tile
from concourse import mybir
from concourse import library_config
from concourse.bass_utils import run_bass_kernel_spmd

F32 = mybir.dt.float32
BF16 = mybir.dt.bfloat16
I16 = mybir.dt.int16
BF = ml_dtypes.bfloat16

NCORES = 8
NNODE = 50000
SPLIT = 32768
DPB = 64                 # dst slots per block
LO_T, HI_T = 5, 3        # low/high-src subtiles per block
TS = LO_T + HI_T         # 8 subtiles per block, 16 per pair-iteration
LOE, HIE = LO_T * 128, HI_T * 128

_built = {}


def _bc(ap, p):
    return bass.AP(tensor=ap.tensor, offset=ap.offset, ap=[[0, p]] + list(ap.ap[1:]))


def _build_L1(B2):
    """proj (u|sk) + conv1 block loop + dense pass emitting kvqs2."""
    NS = B2 * 128
    nc = bacc.Bacc(num_swdge_queues=4)
    xT = nc.declare_dram_parameter("xT", [64, NS], BF16, isOutput=False)
    W1 = nc.declare_dram_parameter("W1", [64, 512], BF16, isOutput=False)
    b1 = nc.declare_dram_parameter("b1", [1, 512], F32, isOutput=False)
    xgA = nc.declare_dram_parameter("xgA", [NNODE, 128], BF16, isOutput=False)
    xgB = nc.declare_dram_parameter("xgB", [NNODE - SPLIT, 128], BF16, isOutput=False)
    idxA = nc.declare_dram_parameter("idxA", [B2, 128, 2 * LOE // 16], I16, isOutput=False)
    idxB = nc.declare_dram_parameter("idxB", [B2, 128, 2 * HIE // 16], I16, isOutput=False)
    Mb = nc.declare_dram_parameter("Mb", [B2, 128, 16 * DPB], BF16, isOutput=False)
    MTb = nc.declare_dram_parameter("MTb", [B2, 64, 16 * 128], BF16, isOutput=False)
    WvBD = nc.declare_dram_parameter("WvBD", [256, 256], BF16, isOutput=False)
    W2 = nc.declare_dram_parameter("W2", [256, 256], BF16, isOutput=False)
    b2 = nc.declare_dram_parameter("b2", [1, 256], F32, isOutput=False)
    idn = nc.declare_dram_parameter("idn", [128, 128], BF16, isOutput=False)
    outt = nc.declare_dram_parameter("outt", [NS, 256], BF16, isOutput=True)
    proj = nc.dram_tensor("proj", [NS, 512], BF16)
    stage = nc.dram_tensor("stage", [NS, 260], BF16)

    # subtile t -> (block parity, is_low);  layout: 0-4 loB, 5-9 loB1,
    # 10-12 hiB, 13-15 hiB1
    blk_of = [0] * LO_T + [1] * LO_T + [0] * HI_T + [1] * HI_T

    with tile.TileContext(nc) as tc:
        nc.gpsimd.load_library(library_config.mlp)
        with tc.tile_pool(name="one", bufs=1) as one:
            W1t = one.tile([64, 512], BF16)
            nc.sync.dma_start(out=W1t[:], in_=W1[:])
            b1t = one.tile([128, 512], F32)
            nc.sync.dma_start(out=b1t[:], in_=_bc(b1[:], 128))
            Wvt = one.tile([128, 2, 256], BF16, name="wvt")
            W2t = one.tile([128, 2, 256], BF16, name="w2t")
            for k in range(2):
                nc.sync.dma_start(out=Wvt[:, k, :], in_=WvBD[k * 128:(k + 1) * 128, :])
                nc.sync.dma_start(out=W2t[:, k, :], in_=W2[k * 128:(k + 1) * 128, :])
            b2t = one.tile([128, 256], F32)
            nc.sync.dma_start(out=b2t[:], in_=_bc(b2[:], 128))
            ident = one.tile([128, 128], BF16)
            nc.sync.dma_start(out=ident[:], in_=idn[:])

            # ---- phase P: proj = x @ [A|Ws] + [bu|bs'] (slot-major) ----
            with tc.tile_pool(name="psb", bufs=3) as sb, \
                 tc.tile_pool(name="pps", bufs=2, space="PSUM") as ps:
                for j in range(B2):
                    xt = sb.tile([64, 128], BF16, tag="xt")
                    nc.sync.dma_start(out=xt[:], in_=xT[:, j * 128:(j + 1) * 128])
                    pp = ps.tile([128, 512], F32, tag="pp")
                    nc.tensor.matmul(out=pp[:], lhsT=xt[:], rhs=W1t[:],
                                     start=True, stop=True)
                    pt = sb.tile([128, 512], BF16, tag="pt")
                    nc.vector.tensor_add(pt[:], pp[:], b1t[:])
                    nc.sync.dma_start(out=proj[j * 128:(j + 1) * 128, :], in_=pt[:])

            # ---- block-pair loop ----
            with tc.tile_pool(name="sb", bufs=2) as sb, \
                 tc.tile_pool(name="qps", bufs=1, space="PSUM") as qps, \
                 tc.tile_pool(name="aps", bufs=2, space="PSUM") as aps:
                for i in range(B2):
                    iA = sb.tile([128, 2 * LOE // 16], I16, tag="iA")
                    nc.sync.dma_start(out=iA[:], in_=idxA[i])
                    iB = sb.tile([128, 2 * HIE // 16], I16, tag="iB")
                    nc.sync.dma_start(out=iB[:], in_=idxB[i])
                    xs = sb.tile([128, 16, 128], BF16, tag="xs")
                    nc.gpsimd.dma_gather(
                        xs[:, 0:2 * LO_T, :], xgA[:], iA[:], 2 * LOE, 2 * LOE,
                        128, single_packet=False, queue_num=(2 * i) % 4)
                    nc.gpsimd.dma_gather(
                        xs[:, 2 * LO_T:16, :], xgB[:], iB[:], 2 * HIE, 2 * HIE,
                        128, single_packet=False, queue_num=(2 * i + 1) % 4)
                    Mt = sb.tile([128, 16, DPB], BF16, tag="Mt")
                    nc.sync.dma_start(out=Mt[:], in_=Mb[i])
                    MTt = sb.tile([64, 16, 128], BF16, tag="MTt")
                    nc.sync.dma_start(out=MTt[:], in_=MTb[i])
                    qr = [sb.tile([64, 256], BF16, tag=f"qr{h}", name=f"qr{h}")
                          for h in range(2)]
                    for h in range(2):
                        nc.sync.dma_start(
                            out=qr[h][:],
                            in_=proj[i * 128 + h * 64:i * 128 + (h + 1) * 64, 0:256])
                    vwe = sb.tile([128, 16, 260], BF16, tag="vwe")
                    prod = sb.tile([128, 16, 4, 64], BF16, tag="prod")
                    qepb = sb.tile([128, 16, 256], BF16, tag="qepb")
                    alph = sb.tile([128, 16, 4], BF16, tag="alph")
                    E4 = sb.tile([128, 16, 4, 64], BF16, tag="E4")
                    for g in range(2):
                        qep = qps.tile([128, 8, 256], F32, tag="qep")
                        for k in range(8):
                            t = g * 8 + k
                            nc.tensor.matmul(out=qep[:, k, :],
                                             lhsT=MTt[:, t, :], rhs=qr[blk_of[t]][:],
                                             start=True, stop=True)
                        sl = slice(g * 8, (g + 1) * 8)
                        nc.scalar.activation(qepb[:, sl], qep[:],
                                             mybir.ActivationFunctionType.Copy)
                        nc.vector.tensor_mul(
                            prod[:, sl],
                            xs[:, sl, 0:64].unsqueeze(2).to_broadcast([128, 8, 4, 64]),
                            qepb[:, sl].rearrange("p k (h d) -> p k h d", h=4))
                        with nc.allow_low_precision(reason="O(1) attn scores"):
                            nc.vector.reduce_sum(
                                out=alph[:, sl], in_=prod[:, sl],
                                axis=mybir.AxisListType.X)
                    nc.scalar.activation(
                        E4[:], alph[:].unsqueeze(3).to_broadcast([128, 16, 4, 64]),
                        mybir.ActivationFunctionType.Exp, scale=0.125)
                    nc.scalar.activation(vwe[:, :, 256:260], alph[:],
                                         mybir.ActivationFunctionType.Exp,
                                         scale=0.125)
                    nc.vector.tensor_mul(
                        vwe[:, :, 0:256].rearrange("p t (h d) -> p t h d", h=4),
                        xs[:, :, 0:64].unsqueeze(2).to_broadcast([128, 16, 4, 64]),
                        E4[:])
                    agg = aps.tile([128, 260], F32, tag="agg")
                    last_t = {0: LO_T * 2 + HI_T - 1, 1: 15}
                    first_t = {0: 0, 1: LO_T}
                    for t in range(16):
                        b = blk_of[t]
                        nc.tensor.matmul(out=agg[b * 64:(b + 1) * 64, :],
                                         lhsT=Mt[:, t, :], rhs=vwe[:, t, :],
                                         start=(t == first_t[b]),
                                         stop=(t == last_t[b]))
                    st = sb.tile([128, 260], BF16, tag="st")
                    nc.vector.tensor_copy(st[:], agg[:])
                    nc.sync.dma_start(out=stage[i * 128:(i + 1) * 128, :], in_=st[:])

            # ---- dense pass: normalize, Wv, skip, relu, W2 ----
            with tc.tile_pool(name="db", bufs=3) as sb, \
                 tc.tile_pool(name="dps", bufs=2, space="PSUM") as ps:
                for i in range(B2):
                    pre = sb.tile([128, 260], BF16, tag="pre")
                    nc.sync.dma_start(out=pre[:], in_=stage[i * 128:(i + 1) * 128, :])
                    sc = sb.tile([128, 4], F32, tag="sc")
                    nc.vector.tensor_scalar_max(sc[:], pre[:, 256:260], 1e-30)
                    rs = sb.tile([128, 4], F32, tag="rs")
                    nc.vector.reciprocal(rs[:], sc[:])
                    zn = sb.tile([128, 256], BF16, tag="zn")
                    nc.vector.tensor_mul(
                        zn[:].rearrange("p (h d) -> p h d", h=4),
                        pre[:, 0:256].rearrange("p (h d) -> p h d", h=4),
                        rs[:].unsqueeze(2).to_broadcast([128, 4, 64]))
                    znTp = ps.tile([128, 256], BF16, tag="znTp")
                    for k in range(2):
                        nc.tensor.transpose(out=znTp[:, k * 128:(k + 1) * 128],
                                            in_=zn[:, k * 128:(k + 1) * 128],
                                            identity=ident[:])
                    znT = sb.tile([128, 256], BF16, tag="znT")
                    nc.vector.tensor_copy(znT[:], znTp[:])
                    h1p = ps.tile([128, 256], F32, tag="h1p")
                    for k in range(2):
                        nc.tensor.matmul(out=h1p[:], lhsT=znT[:, k * 128:(k + 1) * 128],
                                         rhs=Wvt[:, k, :],
                                         start=(k == 0), stop=(k == 1))
                    skt = sb.tile([128, 256], BF16, tag="skt")
                    nc.sync.dma_start(out=skt[:],
                                      in_=proj[i * 128:(i + 1) * 128, 256:512])
                    hb = sb.tile([128, 256], BF16, tag="hb")
                    nc.vector.tensor_add(hb[:], h1p[:], skt[:])
                    nc.scalar.activation(hb[:], hb[:],
                                         mybir.ActivationFunctionType.Relu)
                    hTp = ps.tile([128, 256], BF16, tag="hTp")
                    for k in range(2):
                        nc.tensor.transpose(out=hTp[:, k * 128:(k + 1) * 128],
                                            in_=hb[:, k * 128:(k + 1) * 128],
                                            identity=ident[:])
                    hT = sb.tile([128, 256], BF16, tag="hT")
                    nc.vector.tensor_copy(hT[:], hTp[:])
                    o2p = ps.tile([128, 256], F32, tag="o2p")
                    for k in range(2):
                        nc.tensor.matmul(out=o2p[:], lhsT=hT[:, k * 128:(k + 1) * 128],
                                         rhs=W2t[:, k, :],
                                         start=(k == 0), stop=(k == 1))
                    ot = sb.tile([128, 256], BF16, tag="ot")
                    nc.vector.tensor_add(ot[:], o2p[:], b2t[:])
                    nc.sync.dma_start(out=outt[i * 128:(i + 1) * 128, :], in_=ot[:])
    nc.finalize()
    return nc


def _build_L2(B2):
    """conv2 (1 head) + final linear."""
    NS = B2 * 128
    nc = bacc.Bacc(num_swdge_queues=4)
    p2 = nc.declare_dram_parameter("p2", [NS, 128], BF16, isOutput=False)   # q2|sk2
    kvA = nc.declare_dram_parameter("kvA", [NNODE, 128], BF16, isOutput=False)
    kvB = nc.declare_dram_parameter("kvB", [NNODE - SPLIT, 128], BF16, isOutput=False)
    idxA = nc.declare_dram_parameter("idxA", [B2, 128, 2 * LOE // 16], I16, isOutput=False)
    idxB = nc.declare_dram_parameter("idxB", [B2, 128, 2 * HIE // 16], I16, isOutput=False)
    Mb = nc.declare_dram_parameter("Mb", [B2, 128, 16 * DPB], BF16, isOutput=False)
    MTb = nc.declare_dram_parameter("MTb", [B2, 64, 16 * 128], BF16, isOutput=False)
    Wl = nc.declare_dram_parameter("Wl", [64, 2], BF16, isOutput=False)
    bl = nc.declare_dram_parameter("bl", [1, 2], F32, isOutput=False)
    idn = nc.declare_dram_parameter("idn", [128, 128], BF16, isOutput=False)
    outf = nc.declare_dram_parameter("outf", [NS, 2], F32, isOutput=True)
    stage = nc.dram_tensor("stage2", [NS, 65], BF16)

    blk_of = [0] * LO_T + [1] * LO_T + [0] * HI_T + [1] * HI_T

    with tile.TileContext(nc) as tc:
        nc.gpsimd.load_library(library_config.mlp)
        with tc.tile_pool(name="one", bufs=1) as one:
            Wlt = one.tile([64, 2], BF16)
            nc.sync.dma_start(out=Wlt[:], in_=Wl[:])
            blt = one.tile([128, 2], F32)
            nc.sync.dma_start(out=blt[:], in_=_bc(bl[:], 128))
            ident = one.tile([128, 128], BF16)
            nc.sync.dma_start(out=ident[:], in_=idn[:])

            with tc.tile_pool(name="sb", bufs=2) as sb, \
                 tc.tile_pool(name="qps", bufs=2, space="PSUM") as qps, \
                 tc.tile_pool(name="aps", bufs=2, space="PSUM") as aps:
                for i in range(B2):
                    iA = sb.tile([128, 2 * LOE // 16], I16, tag="iA")
                    nc.sync.dma_start(out=iA[:], in_=idxA[i])
                    iB = sb.tile([128, 2 * HIE // 16], I16, tag="iB")
                    nc.sync.dma_start(out=iB[:], in_=idxB[i])
                    kvt = sb.tile([128, 16, 128], BF16, tag="kvt")
                    nc.gpsimd.dma_gather(
                        kvt[:, 0:2 * LO_T, :], kvA[:], iA[:], 2 * LOE, 2 * LOE,
                        128, single_packet=False, queue_num=(2 * i) % 4)
                    nc.gpsimd.dma_gather(
                        kvt[:, 2 * LO_T:16, :], kvB[:], iB[:], 2 * HIE, 2 * HIE,
                        128, single_packet=False, queue_num=(2 * i + 1) % 4)
                    Mt = sb.tile([128, 16, DPB], BF16, tag="Mt")
                    nc.sync.dma_start(out=Mt[:], in_=Mb[i])
                    MTt = sb.tile([64, 16, 128], BF16, tag="MTt")
                    nc.sync.dma_start(out=MTt[:], in_=MTb[i])
                    qr = [sb.tile([64, 128], BF16, tag=f"qr{h}", name=f"qr{h}")
                          for h in range(2)]
                    for h in range(2):
                        nc.sync.dma_start(
                            out=qr[h][:],
                            in_=p2[i * 128 + h * 64:i * 128 + (h + 1) * 64, :])
                    vwe = sb.tile([128, 16, 65], BF16, tag="vwe")
                    prod = sb.tile([128, 16, 64], BF16, tag="prod")
                    qepb = sb.tile([128, 16, 64], BF16, tag="qepb")
                    alph = sb.tile([128, 16, 1], BF16, tag="alph")
                    E4 = sb.tile([128, 16, 64], BF16, tag="E4")
                    qep = qps.tile([128, 16, 64], F32, tag="qep")
                    for t in range(16):
                        nc.tensor.matmul(out=qep[:, t, :], lhsT=MTt[:, t, :],
                                         rhs=qr[blk_of[t]][:, 0:64],
                                         start=True, stop=True)
                    nc.scalar.activation(qepb[:], qep[:],
                                         mybir.ActivationFunctionType.Copy)
                    nc.vector.tensor_mul(prod[:], kvt[:, :, 0:64], qepb[:])
                    with nc.allow_low_precision(reason="O(1) attn scores"):
                        nc.vector.reduce_sum(out=alph[:], in_=prod[:],
                                             axis=mybir.AxisListType.X)
                    nc.scalar.activation(
                        E4[:], alph[:].to_broadcast([128, 16, 64]),
                        mybir.ActivationFunctionType.Exp, scale=0.125)
                    nc.scalar.activation(vwe[:, :, 64:65], alph[:],
                                         mybir.ActivationFunctionType.Exp,
                                         scale=0.125)
                    nc.vector.tensor_mul(
                        vwe[:, :, 0:64], kvt[:, :, 64:128], E4[:])
                    agg = aps.tile([128, 65], F32, tag="agg")
                    last_t = {0: LO_T * 2 + HI_T - 1, 1: 15}
                    first_t = {0: 0, 1: LO_T}
                    for t in range(16):
                        b = blk_of[t]
                        nc.tensor.matmul(out=agg[b * 64:(b + 1) * 64, :],
                                         lhsT=Mt[:, t, :], rhs=vwe[:, t, :],
                                         start=(t == first_t[b]),
                                         stop=(t == last_t[b]))
                    st = sb.tile([128, 65], BF16, tag="st")
                    nc.vector.tensor_copy(st[:], agg[:])
                    nc.sync.dma_start(out=stage[i * 128:(i + 1) * 128, :], in_=st[:])

            with tc.tile_pool(name="db", bufs=3) as sb, \
                 tc.tile_pool(name="dps", bufs=2, space="PSUM") as ps:
                for i in range(B2):
                    pre = sb.tile([128, 65], BF16, tag="pre")
                    nc.sync.dma_start(out=pre[:], in_=stage[i * 128:(i + 1) * 128, :])
                    sc = sb.tile([128, 1], F32, tag="sc")
                    nc.vector.tensor_scalar_max(sc[:], pre[:, 64:65], 1e-30)
                    rs = sb.tile([128, 1], F32, tag="rs")
                    nc.vector.reciprocal(rs[:], sc[:])
                    sk2 = sb.tile([128, 64], BF16, tag="sk2")
                    nc.sync.dma_start(out=sk2[:],
                                      in_=p2[i * 128:(i + 1) * 128, 64:128])
                    h2 = sb.tile([128, 64], BF16, tag="h2")
                    nc.vector.tensor_mul(h2[:], pre[:, 0:64],
                                         rs[:].to_broadcast([128, 64]))
                    nc.vector.tensor_add(h2[:], h2[:], sk2[:])
                    nc.scalar.activation(h2[:], h2[:],
                                         mybir.ActivationFunctionType.Relu)
                    hTp = ps.tile([64, 128], BF16, tag="hTp")
                    nc.tensor.transpose(out=hTp[:], in_=h2[:], identity=ident[:])
                    hT = sb.tile([64, 128], BF16, tag="hT")
                    nc.vector.tensor_copy(hT[:], hTp[:])
                    op = ps.tile([128, 2], F32, tag="op")
                    nc.tensor.matmul(out=op[:], lhsT=hT[:], rhs=Wlt[:],
                                     start=True, stop=True)
                    oo = sb.tile([128, 2], F32, tag="oo")
                    nc.vector.tensor_add(oo[:], op[:], blt[:])
                    nc.sync.dma_start(out=outf[i * 128:(i + 1) * 128, :], in_=oo[:])
    nc.finalize()
    return nc


def _wrap_idx(v, n):
    """[n] int16 -> [128, n//16] dma_gather layout (16-wrap, 8x replicated)."""
    assert v.shape[0] == n and n % 16 == 0
    m = v.reshape(n // 16, 16).T                    # [16, n//16]
    return np.tile(m, (8, 1)).astype(np.int16)      # [128, n//16]


def _prep(edge_index):
    """Sort edges by dst, split low/high src, pack into block pairs."""
    src = np.ascontiguousarray(edge_index[0]).astype(np.int64)
    dst = np.ascontiguousarray(edge_index[1]).astype(np.int64)
    E = src.shape[0]
    # sort by dst; within each dst, low srcs (< SPLIT) first
    comb = np.lexsort((src >= SPLIT, dst))
    s_sorted = src[comb]
    d_sorted = dst[comb]
    deg = np.bincount(d_sorted, minlength=NNODE)
    nlo = np.bincount(d_sorted[s_sorted < SPLIT], minlength=NNODE)
    nhi = deg - nlo
    cume = np.concatenate([[0], np.cumsum(deg)])
    targets = [round(E * c / NCORES) for c in range(1, NCORES)]
    nb = [0] + [int(np.searchsorted(cume, t)) for t in targets] + [NNODE]

    cores = []
    for c in range(NCORES):
        n0, n1 = nb[c], nb[c + 1]
        blocks, cur, lo, hi = [], [], 0, 0
        for n in range(n0, n1):
            gl, gh = int(nlo[n]), int(nhi[n])
            assert gl <= LOE and gh <= HIE
            if len(cur) >= DPB or lo + gl > LOE or hi + gh > HIE:
                blocks.append(cur)
                cur, lo, hi = [], 0, 0
            cur.append(n)
            lo += gl
            hi += gh
        if cur:
            blocks.append(cur)
        if len(blocks) % 2:
            blocks.append([])
        cores.append((n0, n1, blocks))
    B2 = max(len(cb) // 2 for _, _, cb in cores)

    per_core = []
    for c in range(NCORES):
        n0, n1, blocks = cores[c]
        idxA = np.zeros((B2, 2 * LOE), np.int64)
        idxB = np.full((B2, 2 * HIE), SPLIT, np.int64)
        Mb = np.zeros((B2, 128, 16 * DPB), BF)
        MTb = np.zeros((B2, 64, 16 * 128), BF)
        slot_node = np.full((B2 * 128,), -1, np.int64)   # slot -> node
        for bi, nodes in enumerate(blocks):
            i, half = bi // 2, bi % 2
            lo_base = half * LOE                 # offset into idxA[i]
            hi_base = half * HIE
            elo, ehi = 0, 0
            for sl, n in enumerate(nodes):
                slot_node[i * 128 + half * 64 + sl] = n
                lo0, hi0 = cume[n], cume[n] + nlo[n]
                for k in range(int(nlo[n])):
                    idxA[i, lo_base + elo + k] = s_sorted[lo0 + k]
                for k in range(int(nhi[n])):
                    idxB[i, hi_base + ehi + k] = s_sorted[hi0 + k]
                elo += int(nlo[n])
                ehi += int(nhi[n])
            # build M/MT for this block from the slot runs
            elo, ehi = 0, 0
            for sl, n in enumerate(nodes):
                for k in range(int(nlo[n])):
                    e = elo + k                   # 0..LOE-1 within block
                    t = half * LO_T + e // 128    # subtile index (0..9)
                    p = e % 128
                    Mb[i, p, t * DPB + sl] = 1.0
                    MTb[i, sl, t * 128 + p] = 1.0
                for k in range(int(nhi[n])):
                    e = ehi + k
                    t = 2 * LO_T + half * HI_T + e // 128   # 10..15
                    p = e % 128
                    Mb[i, p, t * DPB + sl] = 1.0
                    MTb[i, sl, t * 128 + p] = 1.0
                elo += int(nlo[n])
                ehi += int(nhi[n])
        idxBw = np.zeros((B2, 128, 2 * HIE // 16), np.int16)
        idxAw = np.zeros((B2, 128, 2 * LOE // 16), np.int16)
        for i in range(B2):
            idxAw[i] = _wrap_idx(idxA[i].astype(np.int16), 2 * LOE)
            idxBw[i] = _wrap_idx((idxB[i] - SPLIT).astype(np.int16), 2 * HIE)
        per_core.append(dict(n0=n0, n1=n1, idxA=idxAw, idxB=idxBw, Mb=Mb,
                             MTb=MTb, slot_node=slot_node))
    return B2, per_core


def kernel(x, edge_index, Wq1, bq1, Wk1, bk1, Wv1, bv1, Ws1, bs1,
           Wq2, bq2, Wk2, bk2, Wv2, bv2, Ws2, bs2, Wl, bl):
    x = np.asarray(x, np.float32)
    B2, per_core = _prep(np.asarray(edge_index))
    NS = B2 * 128

    if ("L1", B2) not in _built:
        _built[("L1", B2)] = _build_L1(B2)
    if ("L2", B2) not in _built:
        _built[("L2", B2)] = _build_L2(B2)

    # host-folded weights
    Wq1, Wk1 = np.asarray(Wq1, np.float32), np.asarray(Wk1, np.float32)
    Wv1, Ws1 = np.asarray(Wv1, np.float32), np.asarray(Ws1, np.float32)
    bq1, bk1 = np.asarray(bq1, np.float32), np.asarray(bk1, np.float32)
    bv1, bs1 = np.asarray(bv1, np.float32), np.asarray(bs1, np.float32)
    Acat = np.zeros((64, 256), np.float32)
    bu = np.zeros((256,), np.float32)
    WvBD = np.zeros((256, 256), np.float32)
    for h in range(4):
        sl = slice(h * 64, (h + 1) * 64)
        Acat[:, sl] = Wq1[:, sl] @ Wk1[:, sl].T
        bu[sl] = bq1[sl] @ Wk1[:, sl].T
        WvBD[sl, sl] = Wv1[:, sl]
    W1cat = np.concatenate([Acat, Ws1], axis=1)          # [64, 512]
    b1cat = np.concatenate([bu, bs1 + bv1])[None, :]     # [1, 512]
    W2cat = np.concatenate([Wk2, Wv2, Wq2, Ws2], axis=1).astype(np.float32)
    b2cat = np.concatenate([bk2, bv2, bq2, bs2])[None, :].astype(np.float32)

    xg = np.zeros((NNODE, 128), BF)
    xg[:, 0:64] = x.astype(BF)
    cids = list(range(NCORES))

    in1 = []
    for c in cids:
        pc = per_core[c]
        sn = pc["slot_node"]
        xT = np.zeros((64, NS), BF)
        valid = sn >= 0
        xT[:, valid] = x[sn[valid]].T.astype(BF)
        in1.append(dict(
            xT=xT, W1=W1cat.astype(BF), b1=b1cat,
            xgA=xg, xgB=np.ascontiguousarray(xg[SPLIT:]),
            idxA=pc["idxA"], idxB=pc["idxB"], Mb=pc["Mb"], MTb=pc["MTb"],
            WvBD=WvBD.astype(BF), W2=W2cat.astype(BF), b2=b2cat,
            idn=np.eye(128, dtype=BF)))
    res1 = run_bass_kernel_spmd(_built[("L1", B2)], in1, cids)
    t1 = res1.exec_time_ns

    # compact h1-projection rows; build kv2 tables and per-core p2
    kv2 = np.zeros((NNODE, 128), BF)
    p2s = []
    for c in cids:
        pc = per_core[c]
        sn = pc["slot_node"]
        valid = sn >= 0
        o = res1.results[c]["outt"]          # [NS, 256] bf16: k2|v2|q2|sk2
        kv2[sn[valid]] = o[valid][:, 0:128]
        p2 = np.zeros((NS, 128), BF)
        p2[valid] = o[valid][:, 128:256]
        p2s.append(p2)

    in2 = []
    for c in cids:
        pc = per_core[c]
        in2.append(dict(
            p2=p2s[c], kvA=kv2, kvB=np.ascontiguousarray(kv2[SPLIT:]),
            idxA=pc["idxA"], idxB=pc["idxB"], Mb=pc["Mb"], MTb=pc["MTb"],
            Wl=np.asarray(Wl, np.float32).astype(BF),
            bl=np.asarray(bl, np.float32)[None, :],
            idn=np.eye(128, dtype=BF)))
    res2 = run_bass_kernel_spmd(_built[("L2", B2)], in2, cids)
    t2 = res2.exec_time_ns

    out = np.zeros((NNODE, 2), np.float32)
    for c in cids:
        pc = per_core[c]
        sn = pc["slot_node"]
        valid = sn >= 0
        out[sn[valid]] = res2.results[c]["outf"][valid]
    kernel.exec_times = (t1, t2)
    return out
